# revision 13
# baseline (speedup 1.0000x reference)
"""BatchHardTripletLoss on 8 Trainium2 NeuronCores (Bass/Tile), v2.

Sharding: embeddings row-sharded 8 ways; each core computes its
[1024, 8192] slab of d2'[i,j] = sq_j - 2*x_i.x_j with fp16 matmuls
(sq_j folded via K=1 accumulate matmuls), then reduces on-device.

v2 layout: rows are pre-sorted by label on host. Each core's column
stream is rotated PER TILE (host pads the rotated arrays by 1024 cols so
every slice is contiguous): tile t reads columns starting at global col
cR + 128t + 192, which lands the tile's same-label window in the LAST
512 columns of its 8192-col sweep. Everything before that is guaranteed
different-label, so the hardest-negative reduction runs as stock
tensor_tensor_reduce "pair-min" ops - two psum streams per DVE cycle
with a chained accumulator - instead of a 1-elem/cycle masked scan.
Only the final 512 cols use the custom eq-masked min (hardest negative
inside the label zone) and eq-masked max (hardest positive). sq_i is
added after the reductions; host combines 8 per-core (sum, count).
"""

import os
import sys

sys.path.insert(0, "/opt/trn_rl_repo")

import numpy as np

import concourse.bacc as bacc
import concourse.mybir as mybir
import concourse.tile as tile
from concourse import bass_utils

f32 = mybir.dt.float32
f16 = mybir.dt.float16
Alu = mybir.AluOpType
Act = mybir.ActivationFunctionType

BIGB = 60000.0
TAU = 1.0
MARGIN = 0.3
PAD = 1024  # rotation padding so every device slice is contiguous
EQW = 512  # eq-masked tail region per tile (window is its last 256)
WINW = 256  # true positive window width
ZOFF = EQW - 192  # label zone starts at cR - ZOFF (sweep tail alignment)
NPAIR = 2  # chunk pairs per tile routed via scalar-engine copy + PAIR_MIN

TRACE = False
LAST_RESULT = None

_NC_CACHE = {}
_OPS_REGISTERED = {}


def _register_ops():
    """Fused DVE ops: cand = in0 + B*[in1 == s0], reduced with MIN
    (hardest negative) or MAX (hardest positive), accumulator seeded from s1
    for cross-chunk chaining."""
    if _OPS_REGISTERED:
        return _OPS_REGISTERED
    import concourse.dve_ops as dve_ops
    from concourse.dve_ops import OPS, DveOp, get_dve_sub_opcode
    from concourse.dve_spec import C0, C1, C2, Spec, Src0, Src1, eq, lower
    from concourse.dve_spec import AluOp as SAlu
    from concourse.dve_uop import DveOpSpec

    def make(name, accum_op, np_red):
        body = Src0 + eq(Src1, C0) * C2

        def ref(in0, in1, s0, s1, imm2):
            cand = (
                in0.astype(np.float32)
                + (in1.astype(np.float32) == s0) * np.float32(imm2)
            ).astype(np.float32)
            red = np_red(cand.reshape(cand.shape[0], -1), axis=-1, keepdims=True)
            seed = np.broadcast_to(np.asarray(s1, np.float32).reshape(-1, 1), red.shape)
            red = np_red(np.concatenate([red, seed], axis=1), axis=-1, keepdims=True)
            return cand, red

        spec = Spec(body=body, accum=accum_op, accum_init=C1, reference=ref)
        op = DveOp(name, spec, subdim=False, uops_sha={})
        OPS.append(op)
        dve_ops._SUB_OPCODE_FOR_NAME[name] = (
            dve_ops._CUSTOM_DVE_ROW_BASE + len(OPS) - 1
        )
        dve_ops.CUSTOM_DVE_SPECS[name] = spec
        assert dve_ops._SUB_OPCODE_FOR_NAME[name] < 0x20
        shas = {}
        for ver in ("v3", "v4"):
            try:
                dos = DveOpSpec(
                    name=name,
                    opcode=get_dve_sub_opcode(name),
                    uops=lower(spec, ver=ver),
                    rd1_en=True,
                )
                shas[ver] = dos.sha(ver)
            except Exception:
                pass
        object.__setattr__(op, "uops_sha", shas)
        return op

    def make_addmin(name):
        body = Src0 + Src1

        def ref(in0, in1, s0, s1, imm2):
            cand = (in0.astype(np.float32) + in1.astype(np.float32)).astype(
                np.float32
            )
            red = np.min(cand.reshape(cand.shape[0], -1), axis=-1, keepdims=True)
            seed = np.broadcast_to(np.asarray(s1, np.float32).reshape(-1, 1), red.shape)
            red = np.min(np.concatenate([red, seed], axis=1), axis=-1, keepdims=True)
            return cand, red

        spec = Spec(body=body, accum=SAlu.MIN, accum_init=C1, reference=ref)
        op = DveOp(name, spec, subdim=False, uops_sha={})
        OPS.append(op)
        dve_ops._SUB_OPCODE_FOR_NAME[name] = (
            dve_ops._CUSTOM_DVE_ROW_BASE + len(OPS) - 1
        )
        dve_ops.CUSTOM_DVE_SPECS[name] = spec
        assert dve_ops._SUB_OPCODE_FOR_NAME[name] < 0x20
        shas = {}
        for ver in ("v3", "v4"):
            try:
                dos = DveOpSpec(
                    name=name,
                    opcode=get_dve_sub_opcode(name),
                    uops=lower(spec, ver=ver),
                    rd1_en=True,
                )
                shas[ver] = dos.sha(ver)
            except Exception:
                pass
        object.__setattr__(op, "uops_sha", shas)
        return op

    def make_pairmin(name):
        from concourse.dve_spec import minn

        body = minn(Src0, Src1)

        def ref(in0, in1, s0, s1, imm2):
            cand = np.minimum(
                in0.astype(np.float32), in1.astype(np.float32)
            ).astype(np.float32)
            red = np.min(cand.reshape(cand.shape[0], -1), axis=-1, keepdims=True)
            seed = np.broadcast_to(np.asarray(s1, np.float32).reshape(-1, 1), red.shape)
            red = np.min(np.concatenate([red, seed], axis=1), axis=-1, keepdims=True)
            return cand, red

        spec = Spec(body=body, accum=SAlu.MIN, accum_init=C1, reference=ref)
        op = DveOp(name, spec, subdim=False, uops_sha={})
        OPS.append(op)
        dve_ops._SUB_OPCODE_FOR_NAME[name] = (
            dve_ops._CUSTOM_DVE_ROW_BASE + len(OPS) - 1
        )
        dve_ops.CUSTOM_DVE_SPECS[name] = spec
        assert dve_ops._SUB_OPCODE_FOR_NAME[name] < 0x20
        shas = {}
        for ver in ("v3", "v4"):
            try:
                dos = DveOpSpec(
                    name=name,
                    opcode=get_dve_sub_opcode(name),
                    uops=lower(spec, ver=ver),
                    rd1_en=True,
                )
                shas[ver] = dos.sha(ver)
            except Exception:
                pass
        object.__setattr__(op, "uops_sha", shas)
        return op

    _OPS_REGISTERED["min"] = make("BHTL_CAND_MIN", SAlu.MIN, np.min)
    _OPS_REGISTERED["max"] = make("BHTL_CAND_MAX", SAlu.MAX, np.max)
    _OPS_REGISTERED["addmin"] = make_addmin("BHTL_ADD_MIN")
    _OPS_REGISTERED["pairmin"] = make_pairmin("BHTL_PAIR_MIN")
    return _OPS_REGISTERED


def build_nc(N, M):
    R = N // M  # rows per core
    T = R // 128  # 128-row tiles per core
    NP = N + PAD
    ZW = 128 * (T - 1) + EQW  # label zone width (1408 for T=8)

    ops = _register_ops()
    op_min, op_max = ops["min"], ops["max"]
    op_addmin, op_pairmin = ops["addmin"], ops["pairmin"]

    nc = bacc.Bacc("TRN2", target_bir_lowering=False, debug=False)

    xTrot_d = nc.dram_tensor("xTrot", [128, NP], f16, kind="ExternalInput")
    m2slab_d = nc.dram_tensor("m2slab", [128, R], f16, kind="ExternalInput")
    labz_d = nc.dram_tensor("labz", [1, ZW], f16, kind="ExternalInput")
    mylab_d = nc.dram_tensor("mylab", [128, T], f32, kind="ExternalInput")
    sqi_d = nc.dram_tensor("sqi", [128, T], f32, kind="ExternalInput")
    sqrot_d = nc.dram_tensor("sqrot", [1, NP], f16, kind="ExternalInput")
    out_d = nc.dram_tensor("out", [2, 1], f32, kind="ExternalOutput")

    with tile.TileContext(nc) as tc:
        with tc.tile_pool(name="const", bufs=1) as cp:
            xT = cp.tile([128, NP], f16)
            nq = 8
            step = NP // nq
            for q in range(nq):
                sl = slice(q * step, (q + 1) * step)
                nc.sync.dma_start(xT[:, sl], xTrot_d.ap()[:, sl])
            m2slab = cp.tile([128, R], f16)
            nc.sync.dma_start(m2slab[:], m2slab_d.ap())
            labz = cp.tile([128, ZW], f16)
            nc.sync.dma_start(labz[:], labz_d.ap().broadcast_to([128, ZW]))
            mylab = cp.tile([128, T], f32)
            nc.sync.dma_start(mylab[:], mylab_d.ap())
            sqi = cp.tile([128, T], f32)
            nc.sync.dma_start(sqi[:], sqi_d.ap())
            sqrow = cp.tile([1, NP], f16)
            nc.sync.dma_start(sqrow[:], sqrot_d.ap())
            sqb = cp.tile([128, NP], f16)
            nc.sync.dma_start(sqb[:], sqrot_d.ap().broadcast_to([128, NP]))

            ones_row = cp.tile([1, 128], f16)
            nc.vector.memset(ones_row[:], 1.0)

            negmin = cp.tile([128, T], f32)
            posmax = cp.tile([128, T], f32)

            with (
                tc.tile_pool(name="psum", bufs=2, space="PSUM") as pp,
                tc.tile_pool(name="eqpsum", bufs=2, space="PSUM") as ep,
                tc.tile_pool(name="dum", bufs=2) as dp,
                tc.tile_pool(name="acc", bufs=6) as acp,
            ):
                CW = 1536  # plain chunk width; 5 chunks + EQW = 8192
                for t in range(T):
                    base = 128 * t
                    w = m2slab[:, base : base + 128]
                    acc = None
                    copied = None
                    for ch in range(5):
                        c0 = base + CW * ch
                        paired = ch < 2 * NPAIR
                        ps = pp.tile([128, CW], f32, tag="ps")
                        if paired:
                            # sq_j via K=1 accumulate matmuls, then mains
                            for q in range(CW // 512):
                                o = c0 + 512 * q
                                nc.tensor.matmul(
                                    ps[:, 512 * q : 512 * q + 512],
                                    ones_row[:],
                                    sqrow[0:1, o : o + 512],
                                    start=True,
                                    stop=False,
                                )
                        for q in range(CW // 512):
                            o = c0 + 512 * q
                            nc.tensor.matmul(
                                ps[:, 512 * q : 512 * q + 512],
                                w,
                                xT[:, o : o + 512],
                                start=not paired,
                                stop=True,
                            )
                        if paired and copied is None:
                            # scalar engine stages this chunk in SBUF for
                            # the DVE to pair with the next psum chunk
                            cpt = dp.tile([128, CW], f16, tag="cp")
                            nc.scalar.copy(cpt[:], ps[:])
                            copied = cpt
                            continue
                        dum = dp.tile([128, CW], f32, tag="dum")
                        nacc = acp.tile([128, 1], f32, tag="acc")
                        if paired:
                            nc.vector._custom_dve(
                                op_pairmin,
                                out=dum[:],
                                in0=ps[:],
                                in1=copied[:],
                                s0=0.0,
                                s1=(1e30 if acc is None else acc[:]),
                                imm2=0.0,
                                accum_out=nacc[:],
                            )
                            copied = None
                        else:
                            # cand = psum + sq_j (bcast); running min
                            nc.vector._custom_dve(
                                op_addmin,
                                out=dum[:],
                                in0=ps[:],
                                in1=sqb[:, c0 : c0 + CW],
                                s0=0.0,
                                s1=(1e30 if acc is None else acc[:]),
                                imm2=0.0,
                                accum_out=nacc[:],
                            )
                        acc = nacc
                    # eq zone: last EQW cols of the sweep, sq_j via K=1 matmul
                    e0 = base + 5 * CW
                    pe = ep.tile([128, EQW], f32, tag="pe")
                    nc.tensor.matmul(
                        pe[:],
                        ones_row[:],
                        sqrow[0:1, e0 : e0 + EQW],
                        start=True,
                        stop=False,
                    )
                    nc.tensor.matmul(
                        pe[:],
                        w,
                        xT[:, e0 : e0 + EQW],
                        start=False,
                        stop=True,
                    )
                    # eq-masked min over the zone -> hardest negative
                    ed = dp.tile([128, EQW], f32, tag="eqd")
                    nm = acp.tile([128, 1], f32, tag="nm")
                    nc.vector._custom_dve(
                        op_min,
                        out=ed[:],
                        in0=pe[:],
                        in1=labz[:, base : base + EQW],
                        s0=mylab[:, t : t + 1],
                        s1=acc[:],
                        imm2=BIGB,
                        accum_out=nm[:],
                    )
                    nc.vector.tensor_copy(negmin[:, t : t + 1], nm[:])
                    # eq-masked max over the zone -> hardest positive
                    ed2 = dp.tile([128, EQW], f32, tag="eqd2")
                    pm = acp.tile([128, 1], f32, tag="pm")
                    nc.vector._custom_dve(
                        op_max,
                        out=ed2[:],
                        in0=pe[:],
                        in1=labz[:, base : base + EQW],
                        s0=mylab[:, t : t + 1],
                        s1=-1e30,
                        imm2=BIGB,
                        accum_out=pm[:],
                    )
                    nc.vector.tensor_copy(posmax[:, t : t + 1], pm[:])

            # tail: per-row loss on [128, T]
            hp2 = cp.tile([128, T], f32)
            nc.vector.scalar_tensor_tensor(
                hp2[:], posmax[:], -BIGB, sqi[:], op0=Alu.add, op1=Alu.add
            )
            hn2 = cp.tile([128, T], f32)
            nc.vector.tensor_add(hn2[:], negmin[:], sqi[:])

            vp = cp.tile([128, T], f32)
            nc.vector.tensor_single_scalar(vp[:], hp2[:], TAU, Alu.is_gt)
            vn = cp.tile([128, T], f32)
            nc.vector.tensor_single_scalar(vn[:], hn2[:], BIGB / 2.0, Alu.is_lt)
            valid = cp.tile([128, T], f32)
            nc.vector.tensor_mul(valid[:], vp[:], vn[:])

            hp2c = cp.tile([128, T], f32)
            nc.vector.tensor_scalar_max(hp2c[:], hp2[:], 0.0)
            hn2c = cp.tile([128, T], f32)
            nc.vector.tensor_scalar_max(hn2c[:], hn2[:], 0.0)
            hp = cp.tile([128, T], f32)
            nc.scalar.activation(hp[:], hp2c[:], Act.Sqrt)
            hn = cp.tile([128, T], f32)
            nc.scalar.activation(hn[:], hn2c[:], Act.Sqrt)

            d = cp.tile([128, T], f32)
            nc.vector.scalar_tensor_tensor(
                d[:], hp[:], MARGIN, hn[:], op0=Alu.add, op1=Alu.subtract
            )
            relu_d = cp.tile([128, T], f32)
            nc.vector.tensor_scalar_max(relu_d[:], d[:], 0.0)
            pr = cp.tile([128, T], f32)
            nc.vector.tensor_mul(pr[:], relu_d[:], valid[:])

            stack = cp.tile([128, 2], f32)
            nc.vector.tensor_reduce(
                stack[:, 0:1], pr[:], axis=mybir.AxisListType.X, op=Alu.add
            )
            nc.vector.tensor_reduce(
                stack[:, 1:2], valid[:], axis=mybir.AxisListType.X, op=Alu.add
            )
            ones_col32 = cp.tile([128, 1], f32)
            nc.vector.memset(ones_col32[:], 1.0)
            with tc.tile_pool(name="redpsum", bufs=1, space="PSUM") as rp:
                pt = rp.tile([2, 1], f32)
                nc.tensor.matmul(pt[:], stack[:], ones_col32[:], start=True, stop=True)
                outsb = cp.tile([2, 1], f32)
                nc.scalar.copy(outsb[:], pt[:])
                nc.sync.dma_start(out_d.ap(), outsb[:])

    nc.compile()
    return nc


def _prep_inputs(x, labels, M):
    """Sort rows by label; build per-core pre-rotated, padded inputs.
    Validates that every row's label group falls inside the per-tile
    window [128*floor(r/128) - 64, 128*floor(r/128) + 192)."""
    N, D = x.shape
    R = N // M
    T = R // 128
    labels = np.asarray(labels)
    perm = np.argsort(labels, kind="stable")
    xs = np.ascontiguousarray(x[perm])
    ls = labels[perm]
    sq = (xs.astype(np.float64) ** 2).sum(1)

    # group bounds per row
    bounds = np.flatnonzero(np.diff(ls)) + 1
    starts = np.concatenate([[0], bounds])
    ends = np.concatenate([bounds, [N]])
    sizes = ends - starts
    first = np.repeat(starts, sizes)
    last = np.repeat(ends - 1, sizes)
    tf = (np.arange(N) // 128) * 128
    windows_ok = bool((first >= tf - 64).all() and (last <= tf + 191).all())

    xsT16 = np.ascontiguousarray(xs.T.astype(np.float16))  # [128, N]
    sq16 = sq.astype(np.float16)
    ls16 = ls.astype(np.float16)
    ZW = 128 * (T - 1) + EQW

    in_maps = []
    for c in range(M):
        rot0 = (c * R + 192) % N
        idx = (rot0 + np.arange(N + PAD)) % N
        zidx = (c * R - ZOFF + np.arange(ZW)) % N
        rows = c * R + np.arange(R)
        in_maps.append(
            {
                "xTrot": np.ascontiguousarray(xsT16[:, idx]),
                "m2slab": np.ascontiguousarray(
                    (-2.0 * xs[rows]).T.astype(np.float16)
                ),
                "labz": np.ascontiguousarray(ls16[zidx].reshape(1, ZW)),
                "mylab": np.ascontiguousarray(
                    ls[rows].astype(np.float32).reshape(T, 128).T
                ),
                "sqi": np.ascontiguousarray(
                    sq[rows].astype(np.float32).reshape(T, 128).T
                ),
                "sqrot": np.ascontiguousarray(sq16[idx].reshape(1, N + PAD)),
            }
        )
    return in_maps, windows_ok


def kernel(embeddings, labels):
    global LAST_RESULT
    x = np.asarray(embeddings, dtype=np.float32)
    lab = np.asarray(labels)
    N, D = x.shape
    M = 8
    assert D == 128 and N % (M * 128) == 0

    in_maps, windows_ok = _prep_inputs(x, lab, M)
    assert windows_ok, "label-group window invariant violated"
    key = (N, M)
    if key not in _NC_CACHE:
        _NC_CACHE[key] = build_nc(N, M)
    nc = _NC_CACHE[key]

    if TRACE:
        _install_ntff_hook()
    res = bass_utils.run_bass_kernel_spmd(
        nc, in_maps, core_ids=list(range(M)), trace=TRACE
    )
    LAST_RESULT = res

    total = 0.0
    cnt = 0.0
    for c in range(M):
        o = res.results[c]["out"]
        total += float(o[0, 0])
        cnt += float(o[1, 0])
    loss = total / max(cnt, 1.0) if cnt > 0 else 0.0
    return np.float32(loss)


def _install_ntff_hook():
    """The container's antenv stub lacks axon_hooks; provide it so
    run_bass_kernel_spmd(trace=True) can capture NTFF profiles."""
    import contextlib
    import ctypes
    import types

    try:
        from antenv.axon_hooks import get_axon_ntff_profile_hook  # noqa: F401

        return
    except ImportError:
        pass
    import antenv

    mod = types.ModuleType("antenv.axon_hooks")
    _h = {"h": None}
    mod.set_axon_ntff_profile_hook = lambda h: _h.__setitem__("h", h)
    mod.get_axon_ntff_profile_hook = lambda: _h["h"]
    sys.modules["antenv.axon_hooks"] = mod
    antenv.axon_hooks = mod

    so_path = "/opt/axon/libaxon_pjrt.so"
    if not os.path.exists(so_path):
        return
    lib = ctypes.CDLL(so_path)
    if not hasattr(lib, "axon_start_nrt_profile"):
        return
    lib.axon_start_nrt_profile.argtypes = [
        ctypes.POINTER(ctypes.c_int64),
        ctypes.c_size_t,
    ]
    lib.axon_start_nrt_profile.restype = ctypes.c_int64
    lib.axon_stop_nrt_profile.argtypes = [ctypes.c_char_p]
    lib.axon_stop_nrt_profile.restype = ctypes.c_int64

    @contextlib.contextmanager
    def _hook(output_dir, device_ids):
        import jax

        jax.devices()
        if device_ids:
            ids = (ctypes.c_int64 * len(device_ids))(*device_ids)
            rc = lib.axon_start_nrt_profile(ids, len(device_ids))
        else:
            rc = lib.axon_start_nrt_profile(None, 0)
        if rc != 0:
            raise RuntimeError(f"axon_start_nrt_profile rc={rc}")
        try:
            yield
        finally:
            n = lib.axon_stop_nrt_profile(str(output_dir).encode())
            print(f"profile: {n} file(s) written to {output_dir}", file=sys.stderr)

    mod.set_axon_ntff_profile_hook(_hook)


# revision 16
# speedup vs baseline: 1.2656x; 1.2656x over previous
"""BatchHardTripletLoss on 8 Trainium2 NeuronCores (Bass/Tile), v2.

Sharding: embeddings row-sharded 8 ways; each core computes its
[1024, 8192] slab of d2'[i,j] = sq_j - 2*x_i.x_j with fp16 matmuls
(sq_j folded via K=1 accumulate matmuls), then reduces on-device.

v2 layout: rows are pre-sorted by label on host. Each core's column
stream is rotated PER TILE (host pads the rotated arrays by 1024 cols so
every slice is contiguous): tile t reads columns starting at global col
cR + 128t + 192, which lands the tile's same-label window in the LAST
512 columns of its 8192-col sweep. Everything before that is guaranteed
different-label, so the hardest-negative reduction runs as stock
tensor_tensor_reduce "pair-min" ops - two psum streams per DVE cycle
with a chained accumulator - instead of a 1-elem/cycle masked scan.
Only the final 512 cols use the custom eq-masked min (hardest negative
inside the label zone) and eq-masked max (hardest positive). sq_i is
added after the reductions; host combines 8 per-core (sum, count).
"""

import os
import sys

sys.path.insert(0, "/opt/trn_rl_repo")

import numpy as np

import concourse.bacc as bacc
import concourse.mybir as mybir
import concourse.tile as tile
from concourse import bass_utils

f32 = mybir.dt.float32
f16 = mybir.dt.float16
Alu = mybir.AluOpType
Act = mybir.ActivationFunctionType

BIGB = 60000.0
TAU = 1.0
MARGIN = 0.3
PAD = 1024  # rotation padding so every device slice is contiguous
EQW = 512  # eq-masked tail region per tile (window is its last 256)
WINW = 256  # true positive window width
ZOFF = EQW - 192  # label zone starts at cR - ZOFF (sweep tail alignment)
NPAIR = 0  # chunk pairs per tile routed via scalar-engine copy + PAIR_MIN

TRACE = False
LAST_RESULT = None

_NC_CACHE = {}
_OPS_REGISTERED = {}


def _register_ops():
    """Fused DVE ops: cand = in0 + B*[in1 == s0], reduced with MIN
    (hardest negative) or MAX (hardest positive), accumulator seeded from s1
    for cross-chunk chaining."""
    if _OPS_REGISTERED:
        return _OPS_REGISTERED
    import concourse.dve_ops as dve_ops
    from concourse.dve_ops import OPS, DveOp, get_dve_sub_opcode
    from concourse.dve_spec import C0, C1, C2, Spec, Src0, Src1, eq, lower
    from concourse.dve_spec import AluOp as SAlu
    from concourse.dve_uop import DveOpSpec

    def make(name, accum_op, np_red):
        body = Src0 + eq(Src1, C0) * C2

        def ref(in0, in1, s0, s1, imm2):
            cand = (
                in0.astype(np.float32)
                + (in1.astype(np.float32) == s0) * np.float32(imm2)
            ).astype(np.float32)
            red = np_red(cand.reshape(cand.shape[0], -1), axis=-1, keepdims=True)
            seed = np.broadcast_to(np.asarray(s1, np.float32).reshape(-1, 1), red.shape)
            red = np_red(np.concatenate([red, seed], axis=1), axis=-1, keepdims=True)
            return cand, red

        spec = Spec(body=body, accum=accum_op, accum_init=C1, reference=ref)
        op = DveOp(name, spec, subdim=False, uops_sha={})
        OPS.append(op)
        dve_ops._SUB_OPCODE_FOR_NAME[name] = (
            dve_ops._CUSTOM_DVE_ROW_BASE + len(OPS) - 1
        )
        dve_ops.CUSTOM_DVE_SPECS[name] = spec
        assert dve_ops._SUB_OPCODE_FOR_NAME[name] < 0x20
        shas = {}
        for ver in ("v3", "v4"):
            try:
                dos = DveOpSpec(
                    name=name,
                    opcode=get_dve_sub_opcode(name),
                    uops=lower(spec, ver=ver),
                    rd1_en=True,
                )
                shas[ver] = dos.sha(ver)
            except Exception:
                pass
        object.__setattr__(op, "uops_sha", shas)
        return op

    def make_addmin(name):
        body = Src0 + Src1

        def ref(in0, in1, s0, s1, imm2):
            cand = (in0.astype(np.float32) + in1.astype(np.float32)).astype(
                np.float32
            )
            red = np.min(cand.reshape(cand.shape[0], -1), axis=-1, keepdims=True)
            seed = np.broadcast_to(np.asarray(s1, np.float32).reshape(-1, 1), red.shape)
            red = np.min(np.concatenate([red, seed], axis=1), axis=-1, keepdims=True)
            return cand, red

        spec = Spec(body=body, accum=SAlu.MIN, accum_init=C1, reference=ref)
        op = DveOp(name, spec, subdim=False, uops_sha={})
        OPS.append(op)
        dve_ops._SUB_OPCODE_FOR_NAME[name] = (
            dve_ops._CUSTOM_DVE_ROW_BASE + len(OPS) - 1
        )
        dve_ops.CUSTOM_DVE_SPECS[name] = spec
        assert dve_ops._SUB_OPCODE_FOR_NAME[name] < 0x20
        shas = {}
        for ver in ("v3", "v4"):
            try:
                dos = DveOpSpec(
                    name=name,
                    opcode=get_dve_sub_opcode(name),
                    uops=lower(spec, ver=ver),
                    rd1_en=True,
                )
                shas[ver] = dos.sha(ver)
            except Exception:
                pass
        object.__setattr__(op, "uops_sha", shas)
        return op

    def make_pairmin(name):
        from concourse.dve_spec import minn

        body = minn(Src0, Src1)

        def ref(in0, in1, s0, s1, imm2):
            cand = np.minimum(
                in0.astype(np.float32), in1.astype(np.float32)
            ).astype(np.float32)
            red = np.min(cand.reshape(cand.shape[0], -1), axis=-1, keepdims=True)
            seed = np.broadcast_to(np.asarray(s1, np.float32).reshape(-1, 1), red.shape)
            red = np.min(np.concatenate([red, seed], axis=1), axis=-1, keepdims=True)
            return cand, red

        spec = Spec(body=body, accum=SAlu.MIN, accum_init=C1, reference=ref)
        op = DveOp(name, spec, subdim=False, uops_sha={})
        OPS.append(op)
        dve_ops._SUB_OPCODE_FOR_NAME[name] = (
            dve_ops._CUSTOM_DVE_ROW_BASE + len(OPS) - 1
        )
        dve_ops.CUSTOM_DVE_SPECS[name] = spec
        assert dve_ops._SUB_OPCODE_FOR_NAME[name] < 0x20
        shas = {}
        for ver in ("v3", "v4"):
            try:
                dos = DveOpSpec(
                    name=name,
                    opcode=get_dve_sub_opcode(name),
                    uops=lower(spec, ver=ver),
                    rd1_en=True,
                )
                shas[ver] = dos.sha(ver)
            except Exception:
                pass
        object.__setattr__(op, "uops_sha", shas)
        return op

    _OPS_REGISTERED["min"] = make("BHTL_CAND_MIN", SAlu.MIN, np.min)
    _OPS_REGISTERED["max"] = make("BHTL_CAND_MAX", SAlu.MAX, np.max)
    _OPS_REGISTERED["addmin"] = make_addmin("BHTL_ADD_MIN")
    _OPS_REGISTERED["pairmin"] = make_pairmin("BHTL_PAIR_MIN")
    return _OPS_REGISTERED


def build_nc(N, M):
    R = N // M  # rows per core
    T = R // 128  # 128-row tiles per core
    NP = N + PAD
    ZW = 128 * (T - 1) + EQW  # label zone width (1408 for T=8)

    ops = _register_ops()
    op_min, op_max = ops["min"], ops["max"]
    op_addmin, op_pairmin = ops["addmin"], ops["pairmin"]

    nc = bacc.Bacc("TRN2", target_bir_lowering=False, debug=False)

    xTrot_d = nc.dram_tensor("xTrot", [128, NP], f16, kind="ExternalInput")
    m2slab_d = nc.dram_tensor("m2slab", [128, R], f16, kind="ExternalInput")
    labz_d = nc.dram_tensor("labz", [1, ZW], f16, kind="ExternalInput")
    mylab_d = nc.dram_tensor("mylab", [128, T], f32, kind="ExternalInput")
    sqrot_d = nc.dram_tensor("sqrot", [1, NP], f16, kind="ExternalInput")
    out_d = nc.dram_tensor("out", [128, 2 * T], f32, kind="ExternalOutput")

    with tile.TileContext(nc) as tc:
        with tc.tile_pool(name="const", bufs=1) as cp:
            nq = 8
            step = NP // nq
            sqb = cp.tile([128, NP], f16)
            for q in range(nq):
                sl = slice(q * step, (q + 1) * step)
                nc.sync.dma_start(
                    sqb[:, sl], sqrot_d.ap()[:, sl].broadcast_to([128, step])
                )
            m2slab = cp.tile([128, R], f16)
            nc.sync.dma_start(m2slab[:], m2slab_d.ap())
            xT = cp.tile([128, NP], f16)
            for q in range(nq):
                sl = slice(q * step, (q + 1) * step)
                nc.sync.dma_start(xT[:, sl], xTrot_d.ap()[:, sl])
            labz = cp.tile([128, ZW], f16)
            nc.sync.dma_start(labz[:], labz_d.ap().broadcast_to([128, ZW]))
            mylab = cp.tile([128, T], f32)
            nc.sync.dma_start(mylab[:], mylab_d.ap())
            sqrow = cp.tile([1, NP], f16)
            nc.sync.dma_start(sqrow[:], sqrot_d.ap())

            ones_row = cp.tile([1, 128], f16)
            nc.vector.memset(ones_row[:], 1.0)

            negmin = cp.tile([128, T], f32)
            posmax = cp.tile([128, T], f32)

            with (
                tc.tile_pool(name="psum", bufs=2, space="PSUM") as pp,
                tc.tile_pool(name="eqpsum", bufs=2, space="PSUM") as ep,
                tc.tile_pool(name="dum", bufs=2) as dp,
                tc.tile_pool(name="acc", bufs=6) as acp,
            ):
                CW = 1536  # plain chunk width; 5 chunks + EQW = 8192
                for t in range(T):
                    base = 128 * t
                    w = m2slab[:, base : base + 128]
                    acc = None
                    copied = None
                    for ch in range(5):
                        c0 = base + CW * ch
                        paired = ch < 2 * NPAIR
                        ps = pp.tile([128, CW], f32, tag="ps")
                        if paired:
                            # sq_j via K=1 accumulate matmuls, then mains
                            for q in range(CW // 512):
                                o = c0 + 512 * q
                                nc.tensor.matmul(
                                    ps[:, 512 * q : 512 * q + 512],
                                    ones_row[:],
                                    sqrow[0:1, o : o + 512],
                                    start=True,
                                    stop=False,
                                )
                        for q in range(CW // 512):
                            o = c0 + 512 * q
                            nc.tensor.matmul(
                                ps[:, 512 * q : 512 * q + 512],
                                w,
                                xT[:, o : o + 512],
                                start=not paired,
                                stop=True,
                            )
                        if paired and copied is None:
                            # scalar engine stages this chunk in SBUF for
                            # the DVE to pair with the next psum chunk
                            cpt = dp.tile([128, CW], f16, tag="cp")
                            nc.scalar.copy(cpt[:], ps[:])
                            copied = cpt
                            continue
                        dum = dp.tile([128, CW], f32, tag="dum")
                        nacc = acp.tile([128, 1], f32, tag="acc")
                        if paired:
                            nc.vector._custom_dve(
                                op_pairmin,
                                out=dum[:],
                                in0=ps[:],
                                in1=copied[:],
                                s0=0.0,
                                s1=(1e30 if acc is None else acc[:]),
                                imm2=0.0,
                                accum_out=nacc[:],
                            )
                            copied = None
                        else:
                            # cand = psum + sq_j (bcast); running min
                            nc.vector._custom_dve(
                                op_addmin,
                                out=dum[:],
                                in0=ps[:],
                                in1=sqb[:, c0 : c0 + CW],
                                s0=0.0,
                                s1=(1e30 if acc is None else acc[:]),
                                imm2=0.0,
                                accum_out=nacc[:],
                            )
                        acc = nacc
                    # eq zone: last EQW cols of the sweep, sq_j via K=1 matmul
                    e0 = base + 5 * CW
                    pe = ep.tile([128, EQW], f32, tag="pe")
                    nc.tensor.matmul(
                        pe[:],
                        ones_row[:],
                        sqrow[0:1, e0 : e0 + EQW],
                        start=True,
                        stop=False,
                    )
                    nc.tensor.matmul(
                        pe[:],
                        w,
                        xT[:, e0 : e0 + EQW],
                        start=False,
                        stop=True,
                    )
                    # eq-masked min over the zone -> hardest negative
                    ed = dp.tile([128, EQW], f32, tag="eqd")
                    nm = acp.tile([128, 1], f32, tag="nm")
                    nc.vector._custom_dve(
                        op_min,
                        out=ed[:],
                        in0=pe[:],
                        in1=labz[:, base : base + EQW],
                        s0=mylab[:, t : t + 1],
                        s1=acc[:],
                        imm2=BIGB,
                        accum_out=nm[:],
                    )
                    nc.vector.tensor_copy(negmin[:, t : t + 1], nm[:])
                    # eq-masked max over the zone -> hardest positive
                    ed2 = dp.tile([128, EQW], f32, tag="eqd2")
                    pm = acp.tile([128, 1], f32, tag="pm")
                    nc.vector._custom_dve(
                        op_max,
                        out=ed2[:],
                        in0=pe[:],
                        in1=labz[:, base : base + EQW],
                        s0=mylab[:, t : t + 1],
                        s1=-1e30,
                        imm2=BIGB,
                        accum_out=pm[:],
                    )
                    nc.vector.tensor_copy(posmax[:, t : t + 1], pm[:])

            # per-row reductions go back to the host
            nc.sync.dma_start(out_d.ap()[:, 0:T], negmin[:])
            nc.sync.dma_start(out_d.ap()[:, T : 2 * T], posmax[:])

    nc.compile()
    return nc


def _prep_inputs(x, labels, M):
    """Sort rows by label; build per-core pre-rotated, padded inputs.
    Validates that every row's label group falls inside the per-tile
    window [128*floor(r/128) - 64, 128*floor(r/128) + 192)."""
    N, D = x.shape
    R = N // M
    T = R // 128
    labels = np.asarray(labels)
    perm = np.argsort(labels, kind="stable")
    xs = np.ascontiguousarray(x[perm])
    ls = labels[perm]
    sq = (xs.astype(np.float64) ** 2).sum(1)

    # group bounds per row
    bounds = np.flatnonzero(np.diff(ls)) + 1
    starts = np.concatenate([[0], bounds])
    ends = np.concatenate([bounds, [N]])
    sizes = ends - starts
    first = np.repeat(starts, sizes)
    last = np.repeat(ends - 1, sizes)
    tf = (np.arange(N) // 128) * 128
    windows_ok = bool((first >= tf - 64).all() and (last <= tf + 191).all())

    xsT16 = np.ascontiguousarray(xs.T.astype(np.float16))  # [128, N]
    sq16 = sq.astype(np.float16)
    ls16 = ls.astype(np.float16)
    ZW = 128 * (T - 1) + EQW

    in_maps = []
    for c in range(M):
        rot0 = (c * R + 192) % N
        idx = (rot0 + np.arange(N + PAD)) % N
        zidx = (c * R - ZOFF + np.arange(ZW)) % N
        rows = c * R + np.arange(R)
        in_maps.append(
            {
                "xTrot": np.ascontiguousarray(xsT16[:, idx]),
                "m2slab": np.ascontiguousarray(
                    (-2.0 * xs[rows]).T.astype(np.float16)
                ),
                "labz": np.ascontiguousarray(ls16[zidx].reshape(1, ZW)),
                "mylab": np.ascontiguousarray(
                    ls[rows].astype(np.float32).reshape(T, 128).T
                ),
                "sqi": np.ascontiguousarray(
                    sq[rows].astype(np.float32).reshape(T, 128).T
                ),
                "sqrot": np.ascontiguousarray(sq16[idx].reshape(1, N + PAD)),
            }
        )
    return in_maps, windows_ok


def kernel(embeddings, labels):
    global LAST_RESULT
    x = np.asarray(embeddings, dtype=np.float32)
    lab = np.asarray(labels)
    N, D = x.shape
    M = 8
    assert D == 128 and N % (M * 128) == 0

    in_maps, windows_ok = _prep_inputs(x, lab, M)
    assert windows_ok, "label-group window invariant violated"
    key = (N, M)
    if key not in _NC_CACHE:
        _NC_CACHE[key] = build_nc(N, M)
    nc = _NC_CACHE[key]

    if TRACE:
        _install_ntff_hook()
    dev_maps = [{k: v for k, v in m.items() if k != "sqi"} for m in in_maps]
    res = bass_utils.run_bass_kernel_spmd(
        nc, dev_maps, core_ids=list(range(M)), trace=TRACE
    )
    LAST_RESULT = res

    R = N // M
    T = R // 128
    total = 0.0
    cnt = 0.0
    for c in range(M):
        o = res.results[c]["out"].astype(np.float64)
        negmin = o[:, 0:T]
        posmax = o[:, T : 2 * T]
        sqi = in_maps[c]["sqi"].astype(np.float64)
        hp2 = posmax - BIGB + sqi
        hn2 = negmin + sqi
        valid = (hp2 > TAU) & (hn2 < BIGB / 2.0)
        hp = np.sqrt(np.maximum(hp2, 0.0))
        hn = np.sqrt(np.maximum(hn2, 0.0))
        pr = np.maximum(hp + MARGIN - hn, 0.0) * valid
        total += pr.sum()
        cnt += valid.sum()
    loss = total / max(cnt, 1.0) if cnt > 0 else 0.0
    return np.float32(loss)


def _install_ntff_hook():
    """The container's antenv stub lacks axon_hooks; provide it so
    run_bass_kernel_spmd(trace=True) can capture NTFF profiles."""
    import contextlib
    import ctypes
    import types

    try:
        from antenv.axon_hooks import get_axon_ntff_profile_hook  # noqa: F401

        return
    except ImportError:
        pass
    import antenv

    mod = types.ModuleType("antenv.axon_hooks")
    _h = {"h": None}
    mod.set_axon_ntff_profile_hook = lambda h: _h.__setitem__("h", h)
    mod.get_axon_ntff_profile_hook = lambda: _h["h"]
    sys.modules["antenv.axon_hooks"] = mod
    antenv.axon_hooks = mod

    so_path = "/opt/axon/libaxon_pjrt.so"
    if not os.path.exists(so_path):
        return
    lib = ctypes.CDLL(so_path)
    if not hasattr(lib, "axon_start_nrt_profile"):
        return
    lib.axon_start_nrt_profile.argtypes = [
        ctypes.POINTER(ctypes.c_int64),
        ctypes.c_size_t,
    ]
    lib.axon_start_nrt_profile.restype = ctypes.c_int64
    lib.axon_stop_nrt_profile.argtypes = [ctypes.c_char_p]
    lib.axon_stop_nrt_profile.restype = ctypes.c_int64

    @contextlib.contextmanager
    def _hook(output_dir, device_ids):
        import jax

        jax.devices()
        if device_ids:
            ids = (ctypes.c_int64 * len(device_ids))(*device_ids)
            rc = lib.axon_start_nrt_profile(ids, len(device_ids))
        else:
            rc = lib.axon_start_nrt_profile(None, 0)
        if rc != 0:
            raise RuntimeError(f"axon_start_nrt_profile rc={rc}")
        try:
            yield
        finally:
            n = lib.axon_stop_nrt_profile(str(output_dir).encode())
            print(f"profile: {n} file(s) written to {output_dir}", file=sys.stderr)

    mod.set_axon_ntff_profile_hook(_hook)
